# revision 8
# baseline (speedup 1.0000x reference)
"""Causal attention block (QKV proj + RoPE + causal SDPA + out proj) on 8
Trainium2 NeuronCores.

Sharding: core c = 4*b + g handles batch b (of 2) and head group g (of 4,
4 heads each).  Each core computes q/k/v for its 4 heads from x[b] and the
matching Wqkv column slices, runs causal SDPA, and contracts its 512
input-channel rows of Wproj, producing a partial proj [2048 tok, 2048 C]
(bf16).  The host sums the 4 partials per batch (the "all-reduce").

All matmul operands are bf16 (1 PE cycle/row at any moving width); PSUM
accumulation stays fp32.  The host pre-quantizes x and the weights to bf16
and pre-arranges them so every DMA is contiguous per partition.  Measured
max-rel error vs the fp32 reference is ~4e-3 (gate: 2e-2).

Design notes:
  Startup: the PE is pre-warmed with dummy N=128 matmuls on a memset tile
  (HAM un-throttles ~3.4us in) while the input DMAs stream in strict
  first-use order (one HW queue completes FIFO): wq head 0, xt panel 0,
  wk head 0, remaining q/k heads interleaved, cos/sin, wv, xt panel 1, wp.
  PSUM (8 banks): tag A = [128, 2x512] fp32 pair tiles x 3 bufs (6 banks),
  tag C = [128, 512] x 2 bufs.  Pair tiles let one engine instruction
  process two banks' worth of matmul output, halving fixed overheads.
  Phase A (QKV + RoPE): per (panel, head) one pair tile holds q (half 0)
  and k (half 1), each a 16-matmul accumulation over the 2048 contraction.
  One DVE pair-copy drains PSUM -> bf16 (the only PSUM reader, so the slot
  recycles fast); ACT does the two rotate-half partition-swap copies
  pair-wide; DVE then does the per-half mul/mul/add against cos/sin'
  (sin pre-negated on the first 64 partitions), writing qT/kT [hd, tok].
  v is computed per 128-token block as [tok, feat] (tag C) and copied to
  SBUF bf16 by ACT.  x is loaded from HBM exactly once.
  Phase B (causal SDPA): per 512-query panel, head-serial.  Scores are
  computed transposed (scT[k, q] = kT-block @ qT) into pair tiles, two key
  blocks per tile; one ACT exp covers both blocks (diagonal pairs exp per
  half with their causal clip; a GpSimd tri-multiply masks the diagonal
  128-block).  e-tiles are accumulated on DVE (bf16); the softmax
  denominator is one small ones-matmul per (head, panel) into a pair half,
  inverted with the fast DVE reciprocal and folded into attn@v output by
  DVE.  attn@v lags scores by DELAY key blocks; the per-head tail is
  deferred into the next head's loop so the PE never waits on it.
  Phase C (proj): transposed orientation -- out[tok, C] = sum_h A_h.T @
  Wproj_h -- so each pair tile gives a [128 tok, 1024 C] output slab: one
  DVE cast, one DMA with 2 KiB DRAM lines (the [C, tok] orientation's
  1 KiB lines cap output DMA at ~170 GB/s and used to stall the kernel
  tail).  Each panel's proj is emitted interleaved into the next panel's
  attention (two pair-units after each head); the host sums partials
  without a transpose.
"""

import sys

if "/opt/trn_rl_repo" not in sys.path:
    sys.path.insert(0, "/opt/trn_rl_repo")

from contextlib import ExitStack

import numpy as np

import concourse.bass as bass  # noqa: F401
import concourse.tile as tile
from concourse import bacc, bass_utils, mybir

F32 = mybir.dt.float32
BF16 = mybir.dt.bfloat16
EXP = mybir.ActivationFunctionType.Exp

B, N, C = 2, 2048, 2048
H = 16  # total heads
HD = C // H  # 128
G = 4  # head groups (cores per batch)
HPG = H // G  # 4 heads per group
P = 128
PA = 512  # phase-A token panel
NPA = N // PA  # 4
PB = 512  # phase-B query panel
NPB = N // PB  # 4
KB = C // P  # 16 contraction blocks
DELAY = 4  # attn@v lag (in jb steps) behind scores
SCALE = float(HD) ** -0.5
ROPE_BASE = 10000.0
NWARM = 64  # HAM pre-warm matmuls

_NC_CACHE = {}


def _emit(ctx, tc, t):
    nc = tc.nc
    vec, sca, gp = nc.vector, nc.scalar, nc.gpsimd
    mm = nc.tensor.matmul

    const = ctx.enter_context(tc.tile_pool(name="const", bufs=1))
    wpool = ctx.enter_context(tc.tile_pool(name="w", bufs=1))
    xpool = ctx.enter_context(tc.tile_pool(name="x", bufs=2))
    qkpool = ctx.enter_context(tc.tile_pool(name="qk", bufs=1))
    vpool = ctx.enter_context(tc.tile_pool(name="v", bufs=1))
    tmp = ctx.enter_context(tc.tile_pool(name="tmp", bufs=3))
    epool = ctx.enter_context(tc.tile_pool(name="e", bufs=5))
    apool = ctx.enter_context(tc.tile_pool(name="acc", bufs=2))
    opool = ctx.enter_context(tc.tile_pool(name="o", bufs=3))
    pout = ctx.enter_context(tc.tile_pool(name="po", bufs=3))
    ps = ctx.enter_context(tc.tile_pool(name="ps", bufs=1, space="PSUM"))

    def pair(name):
        return ps.tile([P, 2, 512], F32, tag="A", bufs=3, name=name)

    # ---- PE pre-warm: dummy matmuls on a memset tile while DMAs stream.
    warm = const.tile([P, 256], BF16, name="warm")
    gp.memset(warm, 0.0)
    pwarm = pair("pwarm")
    for i in range(NWARM):
        mm(pwarm[:, 0, 0:128], warm[:, 0:128], warm[:, 128:256],
           start=(i == 0), stop=(i == NWARM - 1))

    # ---- input DMAs in strict first-use order (one queue = FIFO) ----
    xT4 = t["xT"].rearrange("p (pan kb tok) -> p pan kb tok", pan=NPA, kb=KB)

    def load_xt(p):
        xt = xpool.tile([P, KB, PA], BF16, tag="x", name=f"xt{p}")
        nc.sync.dma_start(xt[:, 0:8], xT4[:, p, 0:8])
        nc.sync.dma_start(xt[:, 8:16], xT4[:, p, 8:16])
        return xt

    wq4 = t["wq"].rearrange("p (h kb f) -> p h kb f", h=HPG, kb=KB)
    wk4 = t["wk"].rearrange("p (h kb f) -> p h kb f", h=HPG, kb=KB)
    wq_sb, wk_sb = [], []
    for h in range(HPG):
        wq_sb.append(wpool.tile([P, KB, HD], BF16, name=f"wq_sb{h}"))
        wk_sb.append(wpool.tile([P, KB, HD], BF16, name=f"wk_sb{h}"))
    nc.sync.dma_start(wq_sb[0], wq4[:, 0])
    xts = [load_xt(0)]
    nc.sync.dma_start(wk_sb[0], wk4[:, 0])
    for h in range(1, HPG):
        nc.sync.dma_start(wq_sb[h], wq4[:, h])
        nc.sync.dma_start(wk_sb[h], wk4[:, h])

    CCH = 2 * PA + 2 * P  # 1280 cols
    consts0 = const.tile([P, CCH], BF16, name="consts0")
    nc.sync.dma_start(consts0, t["consts"][:, 0:CCH])

    wv_sb = wpool.tile([P, KB, 512], BF16, name="wv_sb")
    wv3 = t["wv"].rearrange("p (kb f) -> p kb f", kb=KB)
    nc.sync.dma_start(wv_sb[:, 0:8], wv3[:, 0:8])
    nc.sync.dma_start(wv_sb[:, 8:16], wv3[:, 8:16])

    consts1 = const.tile([P, 3 * 2 * PA], BF16, name="consts1")
    nc.sync.dma_start(consts1, t["consts"][:, CCH:])

    xts.append(load_xt(1))

    wp_sb = wpool.tile([P, HPG, N], BF16, name="wp_sb")
    nc.sync.dma_start(wp_sb, t["wp"].rearrange("p (h o) -> p h o", h=HPG))

    tri = consts0[:, 2 * PA : 2 * PA + P]
    ones = consts0[:, 2 * PA + P : 2 * PA + 2 * P]

    def cos_sl(p):
        if p == 0:
            return consts0[:, 0:PA]
        return consts1[:, 2 * PA * (p - 1) : 2 * PA * (p - 1) + PA]

    def sin_sl(p):
        if p == 0:
            return consts0[:, PA : 2 * PA]
        return consts1[:, 2 * PA * (p - 1) + PA : 2 * PA * p]

    qT = [qkpool.tile([P, N], BF16, name=f"qT{h}") for h in range(HPG)]
    kT = [qkpool.tile([P, N], BF16, name=f"kT{h}") for h in range(HPG)]
    v_sb = vpool.tile([P, KB, 512], BF16, name="v_sb")

    # ---- phase A: QKV + RoPE, single sweep ----
    def emit_qk(p, h, xt):
        # q into pair half 0, k into half 1; one DVE copy drains both banks
        pqk = pair(f"pqk{h}")
        for kb in range(KB):
            mm(pqk[:, 0], wq_sb[h][:, kb], xt[:, kb],
               start=(kb == 0), stop=(kb == KB - 1))
        for kb in range(KB):
            mm(pqk[:, 1], wk_sb[h][:, kb], xt[:, kb],
               start=(kb == 0), stop=(kb == KB - 1))
        # rope(t) = t*cos + swap64(t)*sin'   (sin' pre-signed on host)
        sl = slice(PA * p, PA * (p + 1))
        raws = tmp.tile([P, 2, PA], BF16, tag="rws", name="raws")
        vec.tensor_copy(raws, pqk)  # sole PSUM reader -> slot recycles fast
        rawsw = tmp.tile([P, 2, PA], BF16, tag="rwsw", name="rawsw")
        sca.copy(rawsw[0:64], raws[64:128])
        sca.copy(rawsw[64:128], raws[0:64])
        for s, dstT in ((0, qT[h]), (1, kT[h])):
            t1 = tmp.tile([P, PA], BF16, tag="rt1", name="t1")
            t2 = tmp.tile([P, PA], BF16, tag="rt2", name="t2")
            vec.tensor_mul(t1, rawsw[:, s], sin_sl(p))
            vec.tensor_mul(t2, raws[:, s], cos_sl(p))
            vec.tensor_add(dstT[:, sl], t2, t1)

    def emit_v(p, xt):
        for tb in range(PA // P):
            pv = ps.tile([P, 512], F32, tag="C", bufs=2, name=f"pv{tb}")
            for kb in range(KB):
                mm(pv, xt[:, kb, 128 * tb : 128 * (tb + 1)], wv_sb[:, kb],
                   start=(kb == 0), stop=(kb == KB - 1))
            sca.copy(v_sb[:, (PA // P) * p + tb, :], pv)

    for p in range(NPA):
        xt = xts[p] if p < 2 else load_xt(p)
        for h in range(HPG):
            emit_qk(p, h, xt)
        emit_v(p, xt)

    # ---- phase B (SDPA) + phase C (proj), interleaved ----
    out_panel = {}
    pending_tail = []

    def flush_tail():
        while pending_tail:
            pending_tail.pop(0)()

    def emit_b_head(Pp, h):
        njb = 4 * Pp + 4
        po = ps.tile([P, PB], F32, tag="C", bufs=2, name=f"po{h}")
        acc = apool.tile([P, PB], BF16, tag=f"acc{h % 2}", name=f"acc{h}")
        es = []

        def emit_av(jj):
            e_t, s, m0 = es[jj]
            mm(po[:, m0:], v_sb[:, jj, 128 * h : 128 * (h + 1)],
               e_t[:, s, m0:], start=(jj == 0), stop=(jj == njb - 1))

        scp = None
        e1 = None
        pn0 = []
        for jb in range(njb):
            td = jb - 4 * Pp
            n0 = 128 * td if td > 0 else 0
            if jb == 1:
                flush_tail()  # prev head's softmax tail: PE has work queued
            if jb >= DELAY:
                emit_av(jb - DELAY)
            s = jb % 2
            if s == 0:
                scp = pair("scp")
                e1 = epool.tile([P, 2, PB], BF16, tag="e", name="e1")
                pn0 = []
            mm(scp[:, s, n0:], kT[h][:, 128 * jb : 128 * (jb + 1)],
               qT[h][:, PB * Pp + n0 : PB * (Pp + 1)])
            pn0.append(n0)
            es.append((e1, s, n0))
            if s == 1:
                if pn0[0] == pn0[1]:
                    # one exp over both key blocks (2 banks, 1024 cols)
                    sca.activation(e1[:, :, pn0[0]:], scp[:, :, pn0[0]:],
                                   EXP, scale=SCALE)
                else:
                    sca.activation(e1[:, 0, pn0[0]:], scp[:, 0, pn0[0]:],
                                   EXP, scale=SCALE)
                    sca.activation(e1[:, 1, pn0[1]:], scp[:, 1, pn0[1]:],
                                   EXP, scale=SCALE)
                for q in (0, 1):
                    jq = jb - 1 + q
                    tdq = jq - 4 * Pp
                    if tdq >= 0:
                        dsl = slice(128 * tdq, 128 * (tdq + 1))
                        gp.tensor_mul(e1[:, q, dsl], e1[:, q, dsl], tri)
                if jb == 1:
                    vec.tensor_copy(acc, e1[:, 0])
                else:
                    vec.tensor_add(acc[:, pn0[0]:], acc[:, pn0[0]:],
                                   e1[:, 0, pn0[0]:])
                vec.tensor_add(acc[:, pn0[1]:], acc[:, pn0[1]:],
                               e1[:, 1, pn0[1]:])
        for jj in range(max(0, njb - DELAY), njb):
            emit_av(jj)

        def tail():
            # rowsum via tiny PE matmul (partition reduce), fast recip, scale
            prs = pair("prs")
            mm(prs[:, 0], ones, acc)
            rcp = apool.tile([P, PB], F32, tag="rcp", name="rcp")
            vec.reciprocal_approx_fast(rcp, prs[:, 0])
            o_t = opool.tile([P, PB], BF16, tag=f"op{h}", name=f"op{h}")
            vec.tensor_mul(o_t, po, rcp)
            out_panel[Pp, h] = o_t

        pending_tail.append(tail)

    def emit_proj_unit(Pp, u):
        # pair-unit u of 8: token block tb = u // 2, 1024-wide C chunk ch =
        # u % 2.  out[tok, C] = sum_h A_h.T @ Wproj_h
        tb, ch = divmod(u, 2)
        pj = pair("pj")
        for s in (0, 1):
            for h in range(HPG):
                mm(pj[:, s],
                   out_panel[Pp, h][:, 128 * tb : 128 * (tb + 1)],
                   wp_sb[:, h, 1024 * ch + 512 * s : 1024 * ch + 512 * (s + 1)],
                   start=(h == 0), stop=(h == HPG - 1))
        o_t = pout.tile([P, 2, PB], BF16, tag="pout", name="pout")
        vec.tensor_copy(o_t, pj)
        nc.sync.dma_start(
            t["proj"][512 * Pp + 128 * tb : 512 * Pp + 128 * (tb + 1),
                      1024 * ch : 1024 * (ch + 1)],
            o_t.rearrange("p a b -> p (a b)"))

    for Pp in range(NPB):
        for h in range(HPG):
            emit_b_head(Pp, h)
            if Pp > 0:
                emit_proj_unit(Pp - 1, 2 * h)
                emit_proj_unit(Pp - 1, 2 * h + 1)
    flush_tail()
    for u in range(8):
        emit_proj_unit(NPB - 1, u)


def build_nc():
    key = (DELAY,)
    if key in _NC_CACHE:
        return _NC_CACHE[key]
    nc = bacc.Bacc("TRN2", target_bir_lowering=False, debug=False)
    t = {}
    t["xT"] = nc.dram_tensor("xT", [P, NPA * KB * PA], BF16, kind="ExternalInput").ap()
    t["wq"] = nc.dram_tensor("wq", [P, HPG * KB * HD], BF16, kind="ExternalInput").ap()
    t["wk"] = nc.dram_tensor("wk", [P, HPG * KB * HD], BF16, kind="ExternalInput").ap()
    t["wv"] = nc.dram_tensor("wv", [P, KB * 512], BF16, kind="ExternalInput").ap()
    t["wp"] = nc.dram_tensor("wp", [P, HPG * N], BF16, kind="ExternalInput").ap()
    t["consts"] = nc.dram_tensor(
        "consts", [P, 2 * N + 2 * P], BF16, kind="ExternalInput").ap()
    t["proj"] = nc.dram_tensor("proj", [N, N], BF16, kind="ExternalOutput").ap()
    with tile.TileContext(nc) as tc, ExitStack() as ctx:
        _emit(ctx, tc, t)
    nc.compile()
    _NC_CACHE[key] = nc
    return nc


def make_in_maps(x, position_ids, Wqkv, Wproj):
    x = np.asarray(x, dtype=np.float32)
    pos = np.asarray(position_ids, dtype=np.float64)
    Wqkv = np.asarray(Wqkv, dtype=np.float32)
    Wproj = np.asarray(Wproj, dtype=np.float32)
    import ml_dtypes

    inv_freq = 1.0 / (
        ROPE_BASE ** (np.arange(0, HD, 2, dtype=np.float32) / HD)
    )  # [64]
    tri = (np.arange(P)[None, :] >= np.arange(P)[:, None]).astype(
        ml_dtypes.bfloat16
    )
    ones = np.ones((P, P), dtype=ml_dtypes.bfloat16)

    in_maps = []
    for c in range(8):
        b, g = divmod(c, G)
        freqs = pos[b].astype(np.float32)[:, None] * inv_freq[None, :]  # [N, 64]
        emb = np.concatenate([freqs, freqs], axis=-1)  # [N, 128]
        cosT = np.ascontiguousarray(np.cos(emb).T).astype(ml_dtypes.bfloat16)
        sinT = np.sin(emb)
        sinT = np.ascontiguousarray(sinT.T)
        sinT[:64] = -sinT[:64]
        sinT = sinT.astype(ml_dtypes.bfloat16)
        # interleaved per-panel layout: [cos_p0|sin_p0|tri|ones|cos_p1|sin_p1|...]
        chunks = [cosT[:, 0:PA], sinT[:, 0:PA], tri, ones]
        for p in range(1, NPA):
            chunks.append(cosT[:, PA * p : PA * (p + 1)])
            chunks.append(sinT[:, PA * p : PA * (p + 1)])
        consts = np.concatenate(chunks, axis=1)
        bf = ml_dtypes.bfloat16

        def warr(w):  # [2048, 512] -> [p, kb*f] contiguous (kb-major)
            return np.ascontiguousarray(
                w.reshape(KB, P, 512).transpose(1, 0, 2).reshape(P, KB * 512)
            ).astype(bf)

        def warr_h(w):  # [2048, 512] -> [p, h*kb*hd] head-major contiguous
            return np.ascontiguousarray(
                w.reshape(KB, P, HPG, HD).transpose(1, 2, 0, 3).reshape(P, -1)
            ).astype(bf)

        # x[b].T is [C, N]; -> [p, panel, kb, tok] flattened
        xTb = x[b].T.reshape(KB, P, NPA, PA).transpose(1, 2, 0, 3).reshape(P, -1)
        in_maps.append(
            {
                "xT": np.ascontiguousarray(xTb).astype(bf),
                "wq": warr_h(Wqkv[:, 512 * g : 512 * (g + 1)]),
                "wk": warr_h(Wqkv[:, 2048 + 512 * g : 2048 + 512 * (g + 1)]),
                "wv": warr(Wqkv[:, 4096 + 512 * g : 4096 + 512 * (g + 1)]),
                "wp": np.ascontiguousarray(
                    Wproj[512 * g : 512 * (g + 1), :]
                    .reshape(HPG, P, N).transpose(1, 0, 2).reshape(P, HPG * N)
                ).astype(bf),
                "consts": consts,
            }
        )
    return in_maps


def kernel(x, position_ids, Wqkv, Wproj, _trace=False, _tmpdir=None):
    nc = build_nc()
    in_maps = make_in_maps(x, position_ids, Wqkv, Wproj)
    res = bass_utils.run_bass_kernel_spmd(
        nc, in_maps, core_ids=list(range(8)), trace=_trace, tmpdir=_tmpdir
    )
    out = np.empty((B, N, C), dtype=np.float32)
    for b in range(B):
        acc = res.results[4 * b]["proj"].astype(np.float32)
        for g in range(1, G):
            acc += res.results[4 * b + g]["proj"].astype(np.float32)
        out[b] = acc
    kernel.last_exec_time_ns = res.exec_time_ns
    kernel.last_results = res
    return out
